# revision 4
# baseline (speedup 1.0000x reference)
"""Multiresolution hash encoding (InstantNGP-style, 2D) on 8 Trainium2 cores.

Strategy: data-parallel over points. Each core gets 1/8 of x plus the full
table, computes all 16 levels for its points, and the host concatenates.

Per level, corner table indices are computed on the DVE with an
fp32-ALU-exact integer chain (all arithmetic intermediates < 2^24;
bit ops full width), using 2^20 = -42, 2^32 = -172032, 2^38 = 441
(mod P=524309) to reduce the 45-bit hash product. Values are fetched with
per-partition indirect DMA gathers (128 rows / instruction).
"""

import sys

sys.path.insert(0, "/opt/trn_rl_repo")

import numpy as np

import concourse.bass as bass
import concourse.tile as tile
from concourse import bacc, mybir
from concourse.bass_utils import run_bass_kernel_spmd

# ---- problem constants (hardcoded from the task spec) ----
NUM_LEVELS = 16
F = 2
PS1 = 19349663
P = 524309  # first prime >= 2^19
N_POINTS = 524288
N_CORES = 8
START_HASH = 6

SCALES = [16 * (2 ** i) for i in range(NUM_LEVELS)]
OFFSETS = [0]
for i in range(NUM_LEVELS):
    res = SCALES[i]
    n = (res + 1) ** 2
    if n > P:
        n = P
    OFFSETS.append(OFFSETS[-1] + n)
TABLE_SIZE = OFFSETS[-1]  # 5594552

# modular identities for the hash reduction
ALU = mybir.AluOpType
F32 = mybir.dt.float32
I32 = mybir.dt.int32
U32 = mybir.dt.uint32

POINTS_PER_CORE = N_POINTS // N_CORES  # 65536
N_CHUNKS = 8  # separate TileContexts (bounds semaphore counts)


def _build(nc, points_per_core, n_chunks):
    x_d = nc.dram_tensor("x", [points_per_core, 2], F32, kind="ExternalInput")
    data_d = nc.dram_tensor("data", [TABLE_SIZE, 2], F32, kind="ExternalOutput" if False else "ExternalInput")
    out_d = nc.dram_tensor("out", [points_per_core, NUM_LEVELS * F], F32, kind="ExternalOutput")

    cpp = points_per_core // n_chunks  # points per chunk
    cols = cpp // 128                  # free-dim columns per partition

    for chunk in range(n_chunks):
        base = chunk * cpp
        with tile.TileContext(nc) as tc:
            with (
                tc.tile_pool(name="io", bufs=2) as io_pool,
                tc.tile_pool(name="tmp", bufs=2) as tmp,
                tc.tile_pool(name="idx", bufs=2) as idxp,
                tc.tile_pool(name="val", bufs=2) as valp,
            ):
                # ---- load x chunk: point n = base + p*cols + k ----
                x_t = io_pool.tile([128, cols, 2], F32, tag="x")
                src = bass.AP(
                    x_d, base * 2,
                    [[cols * 2, 128], [2, cols], [1, 2]],
                )
                nc.sync.dma_start(out=x_t[:], in_=src)
                out_t = io_pool.tile([128, cols, NUM_LEVELS * F], F32, tag="out")

                def ts(in_ap, s1, op0, s2=None, op1=None, dtype=I32, tag="t"):
                    t = tmp.tile([128, cols], dtype, tag=tag)
                    kw = {}
                    if op1 is not None:
                        kw["op1"] = op1
                    nc.vector.tensor_scalar(
                        out=t[:], in0=in_ap, scalar1=s1, scalar2=s2, op0=op0, **kw
                    )
                    return t

                def tt(a, b, op, dtype=I32, tag="t"):
                    t = tmp.tile([128, cols], dtype, tag=tag)
                    nc.vector.tensor_tensor(out=t[:], in0=a, in1=b, op=op)
                    return t

                def cast(in_ap, dtype, tag="t"):
                    t = tmp.tile([128, cols], dtype, tag=tag)
                    nc.vector.tensor_copy(t[:], in_ap)
                    return t

                def floor_frac(coord_ap, res, axis):
                    """returns (ix int32 tile, frac f32 tile)"""
                    fx = ts(coord_ap, float(res), ALU.mult, dtype=F32, tag=f"fx{axis}")
                    ixr = cast(fx[:], I32, tag=f"ixr{axis}")          # round-nearest
                    fxr = cast(ixr[:], F32, tag=f"fxr{axis}")
                    d = tt(fx[:], fxr[:], ALU.subtract, dtype=F32, tag=f"d{axis}")
                    neg = ts(d[:], 0.0, ALU.is_lt, dtype=F32, tag=f"neg{axis}")
                    negi = cast(neg[:], I32, tag=f"negi{axis}")
                    ix = tt(ixr[:], negi[:], ALU.subtract, tag=f"ix{axis}")
                    frac = tt(d[:], neg[:], ALU.add, dtype=F32, tag=f"frac{axis}")
                    return ix, frac

                def modreduce(m, off, tag):
                    """m int32 tile in (-2^24, 2^24) -> (m mod P) + off"""
                    mf = cast(m[:], F32, tag=tag + "mf")
                    qf = ts(mf[:], float(1.0 / P), ALU.mult, dtype=F32, tag=tag + "qf")
                    q = cast(qf[:], I32, tag=tag + "q")
                    w2 = ts(q[:], P, ALU.mult, -P, ALU.add, tag=tag + "w2")
                    r = tt(m[:], w2[:], ALU.subtract, tag=tag + "r")   # in (0, 2P)
                    rp = ts(r[:], -P, ALU.add, tag=tag + "rp")
                    rf = tmp.tile([128, cols], I32, tag=tag + "rf")
                    nc.vector.tensor_tensor(
                        out=rf[:].bitcast(U32), in0=r[:].bitcast(U32),
                        in1=rp[:].bitcast(U32), op=ALU.min,
                    )
                    if off:
                        rf2 = ts(rf[:], off, ALU.add, tag=tag + "ro")
                        return rf2
                    return rf

                for lvl in range(NUM_LEVELS):
                    res = SCALES[lvl]
                    res1 = res + 1
                    off_l = OFFSETS[lvl]
                    ix, fracx = floor_frac(x_t[:, :, 0], res, "x")
                    iy, fracy = floor_frac(x_t[:, :, 1], res, "y")
                    # corner weights
                    wx0 = ts(fracx[:], -1.0, ALU.mult, 1.0, ALU.add, dtype=F32, tag="wx0")
                    wy0 = ts(fracy[:], -1.0, ALU.mult, 1.0, ALU.add, dtype=F32, tag="wy0")
                    w00 = tt(wx0[:], wy0[:], ALU.mult, dtype=F32, tag="w00")
                    w01 = tt(wx0[:], fracy[:], ALU.mult, dtype=F32, tag="w01")
                    w10 = tt(fracx[:], wy0[:], ALU.mult, dtype=F32, tag="w10")
                    w11 = tt(fracx[:], fracy[:], ALU.mult, dtype=F32, tag="w11")

                    if lvl < START_HASH:
                        # dense: row pair (ix,iy),(ix,iy+1) contiguous -> 16B gather
                        t0 = ts(ix[:], res1, ALU.mult, off_l, ALU.add, tag="ga")
                        ind00 = tt(t0[:], iy[:], ALU.add, tag="ind00")
                        ind10 = ts(ind00[:], res1, ALU.add, tag="ind10")
                        va = valp.tile([128, cols, 4], F32, tag="va")
                        vb = valp.tile([128, cols, 4], F32, tag="vb")
                        for k in range(cols):
                            nc.gpsimd.indirect_dma_start(
                                out=va[:, k, :], out_offset=None, in_=data_d[:],
                                in_offset=bass.IndirectOffsetOnAxis(ap=ind00[:, k:k + 1], axis=0),
                            )
                            nc.gpsimd.indirect_dma_start(
                                out=vb[:, k, :], out_offset=None, in_=data_d[:],
                                in_offset=bass.IndirectOffsetOnAxis(ap=ind10[:, k:k + 1], axis=0),
                            )
                        pieces = [
                            (w00, va, 0), (w01, va, 2), (w10, vb, 0), (w11, vb, 2),
                        ]
                    else:
                        # hashed: exact (ix0 ^ (iy*PS1)) % P via limb chain
                        a = ts(iy[:], 12, ALU.logical_shift_right, tag="ha")
                        b = ts(iy[:], 4095, ALU.bitwise_and, tag="hb")
                        A1 = ts(a[:], 628, ALU.mult, tag="A1")
                        A2 = ts(b[:], 628, ALU.mult, tag="A2")
                        A3 = ts(a[:], 159, ALU.mult, tag="A3")
                        A4 = ts(b[:], 159, ALU.mult, tag="A4")
                        M = tt(A2[:], A3[:], ALU.add, tag="M")
                        M1 = ts(M[:], 8, ALU.logical_shift_right, tag="M1")
                        M0 = ts(M[:], 255, ALU.bitwise_and, tag="M0")
                        Sh = ts(M0[:], 4096, ALU.mult, tag="Sh")
                        S = tt(Sh[:], A4[:], ALU.add, tag="S")
                        S1 = ts(S[:], 20, ALU.logical_shift_right, tag="S1")
                        S0 = ts(S[:], 0xFFFFF, ALU.bitwise_and, tag="S0")
                        G = tt(iy[:], A1[:], ALU.add, tag="G")
                        T = tt(M1[:], S1[:], ALU.add, tag="T")
                        Hh = ts(G[:], 8, ALU.logical_shift_right, tag="Hh")
                        Gl = ts(G[:], 255, ALU.bitwise_and, tag="Gl")
                        Gm = ts(Gl[:], 16, ALU.mult, tag="Gm")
                        Hu = tt(Gm[:], T[:], ALU.add, tag="Hu")
                        Hh1 = ts(Hh[:], 6, ALU.logical_shift_right, tag="Hh1")
                        Hh0 = ts(Hh[:], 63, ALU.bitwise_and, tag="Hh0")
                        z1 = ts(Hu[:], -42, ALU.mult, tag="z1")
                        z2 = ts(Hh1[:], 441, ALU.mult, tag="z2")
                        z3 = ts(Hh0[:], -172032, ALU.mult, tag="z3")
                        z4 = tt(z1[:], z2[:], ALU.add, tag="z4")
                        z = tt(z4[:], z3[:], ALU.add, tag="z")
                        # iy+1 incremental: S0b, zb
                        St = ts(S0[:], PS1 & 0xFFFFF, ALU.add, tag="St")
                        cb = ts(St[:], 20, ALU.logical_shift_right, tag="cb")
                        S0b = ts(St[:], 0xFFFFF, ALU.bitwise_and, tag="S0b")
                        Hub_ = ts(Hu[:], PS1 >> 20, ALU.add, tag="Hub_")
                        Hub = tt(Hub_[:], cb[:], ALU.add, tag="Hub")
                        zb1 = ts(Hub[:], -42, ALU.mult, tag="zb1")
                        zb2 = tt(zb1[:], z2[:], ALU.add, tag="zb2")
                        zb = tt(zb2[:], z3[:], ALU.add, tag="zb")
                        ixp = ts(ix[:], 1, ALU.add, tag="ixp")

                        w_00 = tt(S0[:], ix[:], ALU.bitwise_xor, tag="x00")
                        w_10 = tt(S0[:], ixp[:], ALU.bitwise_xor, tag="x10")
                        w_01 = tt(S0b[:], ix[:], ALU.bitwise_xor, tag="x01")
                        w_11 = tt(S0b[:], ixp[:], ALU.bitwise_xor, tag="x11")
                        m00 = tt(w_00[:], z[:], ALU.add, tag="m00")
                        m10 = tt(w_10[:], z[:], ALU.add, tag="m10")
                        m01 = tt(w_01[:], zb[:], ALU.add, tag="m01")
                        m11 = tt(w_11[:], zb[:], ALU.add, tag="m11")
                        r00 = modreduce(m00, off_l, "r00")
                        r01 = modreduce(m01, off_l, "r01")
                        r10 = modreduce(m10, off_l, "r10")
                        r11 = modreduce(m11, off_l, "r11")

                        vc = [valp.tile([128, cols, 2], F32, tag=f"vc{c}", name=f"vc{c}") for c in range(4)]
                        for k in range(cols):
                            for c, rr in enumerate([r00, r01, r10, r11]):
                                nc.gpsimd.indirect_dma_start(
                                    out=vc[c][:, k, :], out_offset=None, in_=data_d[:],
                                    in_offset=bass.IndirectOffsetOnAxis(ap=rr[:, k:k + 1], axis=0),
                                )
                        pieces = [
                            (w00, vc[0], 0), (w01, vc[1], 0), (w10, vc[2], 0), (w11, vc[3], 0),
                        ]

                    # ---- interpolate: out[:, :, 2l:2l+2] = sum_c w_c * val_c ----
                    prods = []
                    for ci, (w, v, o) in enumerate(pieces):
                        wb = w[:].rearrange("p (k o) -> p k o", o=1).broadcast_to([128, cols, 2])
                        prod = tmp.tile([128, cols, 2], F32, tag=f"prod{ci}")
                        nc.vector.tensor_tensor(out=prod[:], in0=v[:, :, o:o + 2], in1=wb, op=ALU.mult)
                        prods.append(prod)
                    s1 = tmp.tile([128, cols, 2], F32, tag="s1")
                    nc.vector.tensor_tensor(out=s1[:], in0=prods[0][:], in1=prods[1][:], op=ALU.add)
                    s2 = tmp.tile([128, cols, 2], F32, tag="s2")
                    nc.vector.tensor_tensor(out=s2[:], in0=prods[2][:], in1=prods[3][:], op=ALU.add)
                    nc.vector.tensor_tensor(
                        out=out_t[:, :, 2 * lvl:2 * lvl + 2], in0=s1[:], in1=s2[:], op=ALU.add,
                    )

                # ---- store out chunk ----
                dst = bass.AP(
                    out_d, base * NUM_LEVELS * F,
                    [[cols * NUM_LEVELS * F, 128], [NUM_LEVELS * F, cols], [1, NUM_LEVELS * F]],
                )
                nc.sync.dma_start(out=dst, in_=out_t[:])
    return nc


_CACHE = {}


def build_kernel(points_per_core=POINTS_PER_CORE, n_chunks=N_CHUNKS):
    key = (points_per_core, n_chunks)
    if key not in _CACHE:
        nc = bacc.Bacc("TRN2", target_bir_lowering=False, debug=False, num_devices=N_CORES)
        _build(nc, points_per_core, n_chunks)
        nc.compile()
        _CACHE[key] = nc
    return _CACHE[key]


def kernel(x: np.ndarray, data: np.ndarray, _trace=False, _points_per_core=POINTS_PER_CORE,
           _n_chunks=N_CHUNKS):
    x = np.ascontiguousarray(x, dtype=np.float32)
    data = np.ascontiguousarray(data, dtype=np.float32)
    nc = build_kernel(_points_per_core, _n_chunks)
    xs = x.reshape(N_CORES, _points_per_core, 2) if _points_per_core * N_CORES == x.shape[0] \
        else np.stack([x[:_points_per_core]] * N_CORES)
    in_maps = [{"x": np.ascontiguousarray(xs[c]), "data": data} for c in range(N_CORES)]
    res = run_bass_kernel_spmd(nc, in_maps, core_ids=list(range(N_CORES)), trace=_trace)
    out = np.concatenate([res.results[c]["out"] for c in range(N_CORES)], axis=0)
    if _points_per_core * N_CORES != x.shape[0]:
        out = out[: x.shape[0]]
    kernel._last_result = res
    return out


# revision 6
# speedup vs baseline: 1.1314x; 1.1314x over previous
"""Multiresolution hash encoding (InstantNGP-style, 2D) on 8 Trainium2 cores.

Strategy: data-parallel over points. Each core gets 1/8 of x plus the full
table, computes all 16 levels for its points, and the host concatenates.

Per level, corner table indices are computed on the DVE with an
fp32-ALU-exact integer chain (all arithmetic intermediates < 2^24;
bit ops full width), using 2^20 = -42, 2^32 = -172032, 2^38 = 441
(mod P=524309) to reduce the 45-bit hash product. Values are fetched with
per-partition indirect DMA gathers (128 rows / instruction).
"""

import sys

sys.path.insert(0, "/opt/trn_rl_repo")

import numpy as np

import concourse.bass as bass
import concourse.tile as tile
from concourse import bacc, mybir
from concourse.bass_utils import run_bass_kernel_spmd

# ---- problem constants (hardcoded from the task spec) ----
NUM_LEVELS = 16
F = 2
PS1 = 19349663
P = 524309  # first prime >= 2^19
N_POINTS = 524288
N_CORES = 8
START_HASH = 6

SCALES = [16 * (2 ** i) for i in range(NUM_LEVELS)]
OFFSETS = [0]
for i in range(NUM_LEVELS):
    res = SCALES[i]
    n = (res + 1) ** 2
    if n > P:
        n = P
    OFFSETS.append(OFFSETS[-1] + n)
TABLE_SIZE = OFFSETS[-1]  # 5594552

# modular identities for the hash reduction
ALU = mybir.AluOpType
F32 = mybir.dt.float32
I32 = mybir.dt.int32
U32 = mybir.dt.uint32

POINTS_PER_CORE = N_POINTS // N_CORES  # 65536
N_CHUNKS = 8  # separate TileContexts (bounds semaphore counts)


def _build(nc, points_per_core, n_chunks):
    x_d = nc.dram_tensor("x", [points_per_core, 2], F32, kind="ExternalInput")
    data_d = nc.dram_tensor("data", [TABLE_SIZE, 2], F32, kind="ExternalOutput" if False else "ExternalInput")
    out_d = nc.dram_tensor("out", [points_per_core, NUM_LEVELS * F], F32, kind="ExternalOutput")

    cpp = points_per_core // n_chunks  # points per chunk
    cols = cpp // 128                  # free-dim columns per partition

    for chunk in range(n_chunks):
        base = chunk * cpp
        with tile.TileContext(nc) as tc:
            with (
                tc.tile_pool(name="io", bufs=2) as io_pool,
                tc.tile_pool(name="tmp", bufs=2) as tmp,
                tc.tile_pool(name="idx", bufs=2) as idxp,
                tc.tile_pool(name="val", bufs=2) as valp,
            ):
                # ---- load x chunk: point n = base + p*cols + k ----
                x_t = io_pool.tile([128, cols, 2], F32, tag="x")
                src = bass.AP(
                    x_d, base * 2,
                    [[cols * 2, 128], [2, cols], [1, 2]],
                )
                nc.sync.dma_start(out=x_t[:], in_=src)
                out_t = io_pool.tile([128, cols, NUM_LEVELS * F], F32, tag="out")

                def ts(in_ap, s1, op0, s2=None, op1=None, dtype=I32, tag="t"):
                    t = tmp.tile([128, cols], dtype, tag=tag)
                    kw = {}
                    if op1 is not None:
                        kw["op1"] = op1
                    nc.vector.tensor_scalar(
                        out=t[:], in0=in_ap, scalar1=s1, scalar2=s2, op0=op0, **kw
                    )
                    return t

                def tt(a, b, op, dtype=I32, tag="t"):
                    t = tmp.tile([128, cols], dtype, tag=tag)
                    nc.vector.tensor_tensor(out=t[:], in0=a, in1=b, op=op)
                    return t

                def cast(in_ap, dtype, tag="t"):
                    t = tmp.tile([128, cols], dtype, tag=tag)
                    nc.vector.tensor_copy(t[:], in_ap)
                    return t

                def floor_frac(coord_ap, res, axis):
                    """returns (ix int32 tile, frac f32 tile)"""
                    fx = ts(coord_ap, float(res), ALU.mult, dtype=F32, tag=f"fx{axis}")
                    ixr = cast(fx[:], I32, tag=f"ixr{axis}")          # round-nearest
                    fxr = cast(ixr[:], F32, tag=f"fxr{axis}")
                    d = tt(fx[:], fxr[:], ALU.subtract, dtype=F32, tag=f"d{axis}")
                    neg = ts(d[:], 0.0, ALU.is_lt, dtype=F32, tag=f"neg{axis}")
                    negi = cast(neg[:], I32, tag=f"negi{axis}")
                    ix = tt(ixr[:], negi[:], ALU.subtract, tag=f"ix{axis}")
                    frac = tt(d[:], neg[:], ALU.add, dtype=F32, tag=f"frac{axis}")
                    return ix, frac

                def modreduce(m, off, tag):
                    """m int32 tile in (-2^24, 2^24) -> (m mod P) + off"""
                    mf = cast(m[:], F32, tag=tag + "mf")
                    qf = ts(mf[:], float(1.0 / P), ALU.mult, dtype=F32, tag=tag + "qf")
                    q = cast(qf[:], I32, tag=tag + "q")
                    w2 = ts(q[:], P, ALU.mult, -P, ALU.add, tag=tag + "w2")
                    r = tt(m[:], w2[:], ALU.subtract, tag=tag + "r")   # in (0, 2P)
                    rp = ts(r[:], -P, ALU.add, tag=tag + "rp")
                    rf = tmp.tile([128, cols], I32, tag=tag + "rf")
                    nc.vector.tensor_tensor(
                        out=rf[:].bitcast(U32), in0=r[:].bitcast(U32),
                        in1=rp[:].bitcast(U32), op=ALU.min,
                    )
                    if off:
                        rf2 = ts(rf[:], off, ALU.add, tag=tag + "ro")
                        return rf2
                    return rf

                for lvl in range(NUM_LEVELS):
                    res = SCALES[lvl]
                    res1 = res + 1
                    off_l = OFFSETS[lvl]
                    ix, fracx = floor_frac(x_t[:, :, 0], res, "x")
                    iy, fracy = floor_frac(x_t[:, :, 1], res, "y")
                    # corner weights
                    wx0 = ts(fracx[:], -1.0, ALU.mult, 1.0, ALU.add, dtype=F32, tag="wx0")
                    wy0 = ts(fracy[:], -1.0, ALU.mult, 1.0, ALU.add, dtype=F32, tag="wy0")
                    w00 = tt(wx0[:], wy0[:], ALU.mult, dtype=F32, tag="w00")
                    w01 = tt(wx0[:], fracy[:], ALU.mult, dtype=F32, tag="w01")
                    w10 = tt(fracx[:], wy0[:], ALU.mult, dtype=F32, tag="w10")
                    w11 = tt(fracx[:], fracy[:], ALU.mult, dtype=F32, tag="w11")

                    if lvl < START_HASH:
                        # dense: one span gather per point covers all 4 corners:
                        # rows ind00 .. ind00+res+2 (corners at +0,+1,+res1,+res1+1)
                        t0 = ts(ix[:], res1, ALU.mult, off_l, ALU.add, tag="ga")
                        ind00 = tt(t0[:], iy[:], ALU.add, tag="ind00")
                        span_e = (res1 + 2) * 2  # f32 elements per span row
                        # col-batch to bound SBUF (span tile bytes/partition)
                        cb_sz = max(1, min(cols, (24 * 1024) // (span_e * 4)))
                        o10 = res1 * 2
                        for cb in range(0, cols, cb_sz):
                            bw = min(cb_sz, cols - cb)
                            sp = valp.tile([128, cb_sz, span_e], F32, tag="sp", name=f"sp{lvl}")
                            for k in range(bw):
                                nc.gpsimd.indirect_dma_start(
                                    out=sp[:, k, :], out_offset=None, in_=data_d[:],
                                    in_offset=bass.IndirectOffsetOnAxis(
                                        ap=ind00[:, cb + k:cb + k + 1], axis=0),
                                )
                            pieces_b = [
                                (w00, sp, 0), (w01, sp, 2),
                                (w10, sp, o10), (w11, sp, o10 + 2),
                            ]
                            prods = []
                            for ci, (w, v, o) in enumerate(pieces_b):
                                wb = w[:, cb:cb + bw].rearrange("p (k o) -> p k o", o=1).broadcast_to([128, bw, 2])
                                prod = tmp.tile([128, cols, 2], F32, tag=f"prod{ci}", name=f"prod{ci}")
                                nc.vector.tensor_tensor(out=prod[:, :bw, :], in0=v[:, :bw, o:o + 2], in1=wb, op=ALU.mult)
                                prods.append(prod)
                            s1 = tmp.tile([128, cols, 2], F32, tag="s1")
                            nc.vector.tensor_tensor(out=s1[:, :bw, :], in0=prods[0][:, :bw, :], in1=prods[1][:, :bw, :], op=ALU.add)
                            s2 = tmp.tile([128, cols, 2], F32, tag="s2")
                            nc.vector.tensor_tensor(out=s2[:, :bw, :], in0=prods[2][:, :bw, :], in1=prods[3][:, :bw, :], op=ALU.add)
                            nc.vector.tensor_tensor(
                                out=out_t[:, cb:cb + bw, 2 * lvl:2 * lvl + 2],
                                in0=s1[:, :bw, :], in1=s2[:, :bw, :], op=ALU.add,
                            )
                        continue
                    else:
                        # hashed: exact (ix0 ^ (iy*PS1)) % P via limb chain
                        a = ts(iy[:], 12, ALU.logical_shift_right, tag="ha")
                        b = ts(iy[:], 4095, ALU.bitwise_and, tag="hb")
                        A1 = ts(a[:], 628, ALU.mult, tag="A1")
                        A2 = ts(b[:], 628, ALU.mult, tag="A2")
                        A3 = ts(a[:], 159, ALU.mult, tag="A3")
                        A4 = ts(b[:], 159, ALU.mult, tag="A4")
                        M = tt(A2[:], A3[:], ALU.add, tag="M")
                        M1 = ts(M[:], 8, ALU.logical_shift_right, tag="M1")
                        M0 = ts(M[:], 255, ALU.bitwise_and, tag="M0")
                        Sh = ts(M0[:], 4096, ALU.mult, tag="Sh")
                        S = tt(Sh[:], A4[:], ALU.add, tag="S")
                        S1 = ts(S[:], 20, ALU.logical_shift_right, tag="S1")
                        S0 = ts(S[:], 0xFFFFF, ALU.bitwise_and, tag="S0")
                        G = tt(iy[:], A1[:], ALU.add, tag="G")
                        T = tt(M1[:], S1[:], ALU.add, tag="T")
                        Hh = ts(G[:], 8, ALU.logical_shift_right, tag="Hh")
                        Gl = ts(G[:], 255, ALU.bitwise_and, tag="Gl")
                        Gm = ts(Gl[:], 16, ALU.mult, tag="Gm")
                        Hu = tt(Gm[:], T[:], ALU.add, tag="Hu")
                        Hh1 = ts(Hh[:], 6, ALU.logical_shift_right, tag="Hh1")
                        Hh0 = ts(Hh[:], 63, ALU.bitwise_and, tag="Hh0")
                        z1 = ts(Hu[:], -42, ALU.mult, tag="z1")
                        z2 = ts(Hh1[:], 441, ALU.mult, tag="z2")
                        z3 = ts(Hh0[:], -172032, ALU.mult, tag="z3")
                        z4 = tt(z1[:], z2[:], ALU.add, tag="z4")
                        z = tt(z4[:], z3[:], ALU.add, tag="z")
                        # iy+1 incremental: S0b, zb
                        St = ts(S0[:], PS1 & 0xFFFFF, ALU.add, tag="St")
                        cb = ts(St[:], 20, ALU.logical_shift_right, tag="cb")
                        S0b = ts(St[:], 0xFFFFF, ALU.bitwise_and, tag="S0b")
                        Hub_ = ts(Hu[:], PS1 >> 20, ALU.add, tag="Hub_")
                        Hub = tt(Hub_[:], cb[:], ALU.add, tag="Hub")
                        zb1 = ts(Hub[:], -42, ALU.mult, tag="zb1")
                        zb2 = tt(zb1[:], z2[:], ALU.add, tag="zb2")
                        zb = tt(zb2[:], z3[:], ALU.add, tag="zb")
                        ixp = ts(ix[:], 1, ALU.add, tag="ixp")

                        w_00 = tt(S0[:], ix[:], ALU.bitwise_xor, tag="x00")
                        w_10 = tt(S0[:], ixp[:], ALU.bitwise_xor, tag="x10")
                        w_01 = tt(S0b[:], ix[:], ALU.bitwise_xor, tag="x01")
                        w_11 = tt(S0b[:], ixp[:], ALU.bitwise_xor, tag="x11")
                        m00 = tt(w_00[:], z[:], ALU.add, tag="m00")
                        m10 = tt(w_10[:], z[:], ALU.add, tag="m10")
                        m01 = tt(w_01[:], zb[:], ALU.add, tag="m01")
                        m11 = tt(w_11[:], zb[:], ALU.add, tag="m11")
                        r00 = modreduce(m00, off_l, "r00")
                        r01 = modreduce(m01, off_l, "r01")
                        r10 = modreduce(m10, off_l, "r10")
                        r11 = modreduce(m11, off_l, "r11")

                        vc = [valp.tile([128, cols, 2], F32, tag=f"vc{c}", name=f"vc{c}") for c in range(4)]
                        for k in range(cols):
                            for c, rr in enumerate([r00, r01, r10, r11]):
                                nc.gpsimd.indirect_dma_start(
                                    out=vc[c][:, k, :], out_offset=None, in_=data_d[:],
                                    in_offset=bass.IndirectOffsetOnAxis(ap=rr[:, k:k + 1], axis=0),
                                )
                        pieces = [
                            (w00, vc[0], 0), (w01, vc[1], 0), (w10, vc[2], 0), (w11, vc[3], 0),
                        ]

                    # ---- interpolate: out[:, :, 2l:2l+2] = sum_c w_c * val_c ----
                    prods = []
                    for ci, (w, v, o) in enumerate(pieces):
                        wb = w[:].rearrange("p (k o) -> p k o", o=1).broadcast_to([128, cols, 2])
                        prod = tmp.tile([128, cols, 2], F32, tag=f"prod{ci}")
                        nc.vector.tensor_tensor(out=prod[:], in0=v[:, :, o:o + 2], in1=wb, op=ALU.mult)
                        prods.append(prod)
                    s1 = tmp.tile([128, cols, 2], F32, tag="s1")
                    nc.vector.tensor_tensor(out=s1[:], in0=prods[0][:], in1=prods[1][:], op=ALU.add)
                    s2 = tmp.tile([128, cols, 2], F32, tag="s2")
                    nc.vector.tensor_tensor(out=s2[:], in0=prods[2][:], in1=prods[3][:], op=ALU.add)
                    nc.vector.tensor_tensor(
                        out=out_t[:, :, 2 * lvl:2 * lvl + 2], in0=s1[:], in1=s2[:], op=ALU.add,
                    )

                # ---- store out chunk ----
                dst = bass.AP(
                    out_d, base * NUM_LEVELS * F,
                    [[cols * NUM_LEVELS * F, 128], [NUM_LEVELS * F, cols], [1, NUM_LEVELS * F]],
                )
                nc.sync.dma_start(out=dst, in_=out_t[:])
    return nc


_CACHE = {}


def build_kernel(points_per_core=POINTS_PER_CORE, n_chunks=N_CHUNKS):
    key = (points_per_core, n_chunks)
    if key not in _CACHE:
        nc = bacc.Bacc("TRN2", target_bir_lowering=False, debug=False, num_devices=N_CORES)
        _build(nc, points_per_core, n_chunks)
        nc.compile()
        _CACHE[key] = nc
    return _CACHE[key]


def kernel(x: np.ndarray, data: np.ndarray, _trace=False, _points_per_core=POINTS_PER_CORE,
           _n_chunks=N_CHUNKS):
    x = np.ascontiguousarray(x, dtype=np.float32)
    data = np.ascontiguousarray(data, dtype=np.float32)
    nc = build_kernel(_points_per_core, _n_chunks)
    xs = x.reshape(N_CORES, _points_per_core, 2) if _points_per_core * N_CORES == x.shape[0] \
        else np.stack([x[:_points_per_core]] * N_CORES)
    in_maps = [{"x": np.ascontiguousarray(xs[c]), "data": data} for c in range(N_CORES)]
    res = run_bass_kernel_spmd(nc, in_maps, core_ids=list(range(N_CORES)), trace=_trace)
    out = np.concatenate([res.results[c]["out"] for c in range(N_CORES)], axis=0)
    if _points_per_core * N_CORES != x.shape[0]:
        out = out[: x.shape[0]]
    kernel._last_result = res
    return out
